# revision 7
# baseline (speedup 1.0000x reference)
"""DCRNN (nn_DCRNN_77257871721177) Trainium2 Bass kernel.

Strategy: data-parallel over batch B=64 across 8 NeuronCores (8 batches
per core), weights/adjacency replicated, zero collectives.  Everything
is SBUF-resident per core.  GRU states are kept feature-major
[feat, (b, node)]; the Chebyshev diffusion terms are precomputed on the
host (P1=A1, P2=2*A1^2-I, P3=A2, P4=2*A2^2-I) so each dconv is 4
independent 512x512 products applied with the node-major x0 as the
stationary operand (output lands feature-major, ready for the
projection GEMM).  Node-major x0 copies are produced with PE
transposes.  Matmuls run in float32r (fp32 data, ~1.4e-4 rel err,
4x faster than plain fp32 on the PE).
"""

import os
import sys

import numpy as np

sys.path.insert(0, "/opt/trn_rl_repo")

T_IN = 12
T_OUT = 12
B = 64
N = 512
NT = 4          # node tiles of 128
HID = 64
M5 = 5          # diffusion matrices (incl. identity)
NCORES = 8
BS = B // NCORES  # batches per core

# layer name -> (Ci, F); feature order in x0 is [state(64), input(Ci)]
LAYERS = {
    "e0": (2, 66),
    "e1": (64, 128),
    "d0": (1, 65),
    "d1": (64, 128),
}

_COMPILED = {}
last_exec_ns = None


def _build(t_in, t_out):
    import contextlib

    import concourse.bacc as bacc
    import concourse.mybir as mybir
    import concourse.tile as tile
    from concourse.masks import make_identity

    F32 = mybir.dt.float32
    F32R = mybir.dt.float32r
    AF = mybir.ActivationFunctionType

    nc = bacc.Bacc()

    # ---------------- DRAM parameters ----------------
    x_nm = nc.declare_dram_parameter("x_nm", [t_in, 128, NT, BS, 2], F32, isOutput=False)
    x_fm = nc.declare_dram_parameter("x_fm", [t_in, 2 * BS, N], F32, isOutput=False)
    pmt = nc.declare_dram_parameter("pmt", [4, 128, NT, N], F32, isOutput=False)

    # x-input projection weights: for e0/d0 these are delta-expanded so the
    # m0-x matmul can use base partition 0 (K spans all batches' channels).
    wparams = {}
    wx_shapes = {"e0": [2 * BS, BS], "d0": [BS, BS], "e1": [HID, 1], "d1": [HID, 1]}
    for lyr, (ci, f) in LAYERS.items():
        for go, o in (("g", 2 * HID), ("c", HID)):
            nm = lyr + go
            ws = wx_shapes[lyr]
            wparams[nm] = (
                nc.declare_dram_parameter(f"w_{nm}", [f, M5, o], F32, isOutput=False),
                nc.declare_dram_parameter(f"wx_{nm}", [ws[0], ws[1], o], F32, isOutput=False),
                nc.declare_dram_parameter(f"b_{nm}", [o, 1], F32, isOutput=False),
            )
    pw = nc.declare_dram_parameter("pw", [HID, 1], F32, isOutput=False)
    pb = nc.declare_dram_parameter("pb", [1, 1], F32, isOutput=False)
    y_out = nc.declare_dram_parameter("y", [t_out, BS, N], F32, isOutput=True)

    with tile.TileContext(nc) as tc:
        ctx = contextlib.ExitStack()
        with ctx:
            const = ctx.enter_context(tc.tile_pool(name="const", bufs=1))
            statep = ctx.enter_context(tc.tile_pool(name="statep", bufs=1))
            x0p = ctx.enter_context(tc.tile_pool(name="x0p", bufs=2))
            xfmp = ctx.enter_context(tc.tile_pool(name="xfmp", bufs=2))
            Xp = ctx.enter_context(tc.tile_pool(name="Xp", bufs=5))
            smallp = ctx.enter_context(tc.tile_pool(name="smallp", bufs=2))
            psX = ctx.enter_context(tc.tile_pool(name="psX", bufs=4, space="PSUM"))
            psO = ctx.enter_context(tc.tile_pool(name="psO", bufs=2, space="PSUM"))
            psT = ctx.enter_context(tc.tile_pool(name="psT", bufs=2, space="PSUM"))

            # ---------------- constants ----------------
            ident = const.tile([128, 128], F32)
            make_identity(nc, ident)

            pmt_sb = []
            for m in range(4):
                pm_t = const.tile([128, NT, N], F32R, name=f"pmt{m}")
                nc.gpsimd.dma_start(out=pm_t[:], in_=pmt[m])
                pmt_sb.append(pm_t)

            wsb = {}
            for nm, (wd, wxd, bd) in wparams.items():
                f, _, o = wd.shape
                w_t = const.tile([f, M5, o], F32R, name=f"w_{nm}")
                nc.gpsimd.dma_start(out=w_t[:], in_=wd[:])
                wx_t = const.tile(list(wxd.shape), F32R, name=f"wx_{nm}")
                nc.gpsimd.dma_start(out=wx_t[:], in_=wxd[:])
                b_t = const.tile([o, 1], F32, name=f"b_{nm}")
                nc.sync.dma_start(out=b_t[:], in_=bd[:])
                wsb[nm] = (w_t, wx_t, b_t)

            pw_t = const.tile([HID, 1], F32R)
            nc.gpsimd.dma_start(out=pw_t[:], in_=pw[:])
            pb_t = const.tile([1, 1], F32)
            nc.sync.dma_start(out=pb_t[:], in_=pb[:])

            zstage = const.tile([HID, N], F32)
            nc.vector.memset(zstage[:], 0.0)

            # ---------------- states ----------------
            h0 = statep.tile([HID, BS, N], F32R)
            h1 = statep.tile([HID, BS, N], F32R)
            rh = statep.tile([HID, BS, N], F32R)
            u_all = statep.tile([HID, BS, N], F32)
            din = statep.tile([BS, N], F32R)
            for b in range(BS):
                nc.vector.tensor_copy(h0[:, b, :], zstage[:, :])
                nc.vector.tensor_copy(h1[:, b, :], zstage[:, :])
            nc.vector.tensor_copy(din[:, :], zstage[0:BS, :])

            def t_state(dst, h, col_off, b=None):
                """dst[:, kt, bb, col_off:col_off+64] = h[:, bb, chunk kt].T"""
                bs = range(BS) if b is None else [b]
                for bb in bs:
                    for kt in range(NT):
                        pst = psT.tile([128, HID], F32, name="pst", tag="pst")
                        nc.tensor.transpose(
                            pst[0:128, 0:HID],
                            h.bitcast(F32)[:, bb, kt * 128 : (kt + 1) * 128],
                            ident[0:HID, 0:HID],
                        )
                        nc.any.tensor_copy(
                            dst[:, kt, bb, col_off : col_off + HID],
                            pst[0:128, 0:HID],
                        )

            def t_din(dst, col_off):
                """dst[:, kt, b, col_off] = din[b, chunk kt].T for all b at once."""
                for kt in range(NT):
                    pst = psT.tile([128, HID], F32, name="pst", tag="pst")
                    nc.tensor.transpose(
                        pst[0:128, 0:BS],
                        din.bitcast(F32)[:, kt * 128 : (kt + 1) * 128],
                        ident[0:BS, 0:BS],
                    )
                    nc.any.tensor_copy(
                        dst[:, kt, :, col_off],
                        pst[0:128, 0:BS],
                    )

            def diffuse_project(x0t, b, f, w_t, o, m0s_rhs, wx_ap, m0x_rhs, psname):
                """One batch's diffusion + projection; returns psum tile [o, N]."""
                Xs = []
                for m in range(4):
                    px = psX.tile([128, N], F32, name="px", tag="px")
                    for kt in range(NT):
                        nc.tensor.matmul(
                            px[0:f, :],
                            x0t[:, kt, b, 0:f],
                            pmt_sb[m][:, kt, :],
                            start=(kt == 0),
                            stop=(kt == NT - 1),
                        )
                    Xm = Xp.tile([128, N], F32R, name="Xm", tag="Xm")
                    nc.vector.tensor_copy(Xm[0:f, :], px[0:f, :])
                    Xs.append(Xm)
                po = psO.tile([128, N], F32, name=psname, tag="psO")
                nc.tensor.matmul(po[0:o, :], w_t[0:HID, 0, :], m0s_rhs, start=True, stop=False)
                nc.tensor.matmul(po[0:o, :], wx_ap, m0x_rhs, start=False, stop=False)
                for m in range(4):
                    nc.tensor.matmul(
                        po[0:o, :], w_t[0:f, m + 1, :], Xs[m][0:f, :],
                        start=False, stop=(m == 3),
                    )
                return po

            def gru_cell(lyr, t, h, xin_kind, xsrc):
                """One DCGRU cell; updates h in place.
                xin_kind: 'dma' (enc0), 'fm' (xsrc = state tile [64,BS,N]),
                'fm1' (decoder input from din)."""
                ci, f = LAYERS[lyr]
                wg_t, wxg_t, bg_t = wsb[lyr + "g"]
                wc_t, wxc_t, bc_t = wsb[lyr + "c"]

                # ---- x0 (node-major stationary); state cols are later
                # overwritten per-batch with (r*h).T for the candidate pass ----
                x0 = x0p.tile([128, NT, BS, 128], F32R, name="x0", tag="x0")
                t_state(x0, h, 0)
                if xin_kind == "dma":
                    nc.gpsimd.dma_start(out=x0[:, :, :, HID : HID + ci], in_=x_nm[t])
                    xfm_t = xfmp.tile([2 * BS, N], F32R, name="xfm", tag="xfm")
                    nc.gpsimd.dma_start(out=xfm_t[:], in_=x_fm[t])
                elif xin_kind == "fm":
                    t_state(x0, xsrc, HID)
                else:  # fm1
                    t_din(x0, HID)

                def x_rhs(b, wx_t):
                    if xin_kind == "dma":
                        return wx_t[:, b, :], xfm_t[:, :]
                    if xin_kind == "fm":
                        return wx_t[:, 0, :], xsrc[:, b, :]
                    return wx_t[:, b, :], din[:, :]

                # ---- gate pass: r/u, then overwrite b's state cols with (r*h).T ----
                for b in range(BS):
                    wx_ap, xr_ap = x_rhs(b, wxg_t)
                    po = diffuse_project(x0, b, f, wg_t, 2 * HID, h[:, b, :], wx_ap, xr_ap, "pog")
                    r_t = smallp.tile([HID, N], F32, name="r_t", tag="r_t")
                    nc.scalar.activation(r_t[:], po[0:HID, :], AF.Sigmoid, bias=bg_t[0:HID, :])
                    u_hi = smallp.tile([128, N], F32, name="u_hi", tag="u_hi")
                    nc.scalar.activation(
                        u_hi[HID:128, :], po[HID : 2 * HID, :], AF.Sigmoid,
                        bias=bg_t[HID : 2 * HID, :],
                    )
                    nc.vector.tensor_copy(u_all[:, b, :], u_hi[HID:128, :])
                    nc.vector.tensor_mul(rh[:, b, :], r_t[:], h.bitcast(F32)[:, b, :])
                    t_state(x0, rh, 0, b=b)

                # ---- candidate pass + combine ----
                for b in range(BS):
                    wx_ap, xr_ap = x_rhs(b, wxc_t)
                    po = diffuse_project(x0, b, f, wc_t, HID, rh[:, b, :], wx_ap, xr_ap, "poc")
                    c_t = smallp.tile([HID, N], F32, name="c_t", tag="c_t")
                    nc.scalar.activation(c_t[:], po[0:HID, :], AF.Tanh, bias=bc_t[:, :])
                    tm = smallp.tile([HID, N], F32, name="tm", tag="r_t")
                    nc.vector.tensor_sub(tm[:], h.bitcast(F32)[:, b, :], c_t[:])
                    nc.vector.tensor_mul(tm[:], u_all[:, b, :], tm[:])
                    nc.vector.tensor_add(h[:, b, :], c_t[:], tm[:])

            # ---------------- encoder ----------------
            for t in range(t_in):
                gru_cell("e0", t, h0, "dma", None)
                gru_cell("e1", t, h1, "fm", h0)

            # ---------------- decoder ----------------
            for t in range(t_out):
                gru_cell("d0", t, h0, "fm1", None)
                gru_cell("d1", t, h1, "fm", h0)
                for b in range(BS):
                    py = psO.tile([1, N], F32, name="py", tag="psO")
                    nc.tensor.matmul(py[0:1, :], pw_t[:, :], h1[:, b, :], start=True, stop=True)
                    yrow = smallp.tile([1, N], F32, name="yrow", tag="yrow")
                    nc.vector.tensor_scalar_add(yrow[:], py[0:1, :], pb_t[:, :])
                    nc.gpsimd.dma_start(out=din[b : b + 1, :], in_=yrow[0:1, :])
                nc.sync.dma_start(out=y_out[t], in_=din.bitcast(F32)[:, :])

    nc.compile()
    return nc


def _get_nc(t_in, t_out):
    key = (t_in, t_out)
    if key not in _COMPILED:
        _COMPILED[key] = _build(t_in, t_out)
    return _COMPILED[key]


def _prep_shared(inputs):
    A = np.asarray(inputs["A"], np.float32)
    eye = np.eye(N, dtype=np.float32)
    P = [A[0], 2.0 * (A[0] @ A[0]) - eye, A[1], 2.0 * (A[1] @ A[1]) - eye]
    pmt = np.ascontiguousarray(
        np.stack([p.T.reshape(NT, 128, N).transpose(1, 0, 2) for p in P])
    )

    shared = {"pmt": pmt}
    names = {
        "e0g": ("e0_gw", "e0_gb", 2), "e0c": ("e0_cw", "e0_cb", 2),
        "e1g": ("e1_gw", "e1_gb", 64), "e1c": ("e1_cw", "e1_cb", 64),
        "d0g": ("d0_gw", "d0_gb", 1), "d0c": ("d0_cw", "d0_cb", 1),
        "d1g": ("d1_gw", "d1_gb", 64), "d1c": ("d1_cw", "d1_cb", 64),
    }
    for nm, (wkey, bkey, ci) in names.items():
        w = np.asarray(inputs[wkey], np.float32)
        bvec = np.asarray(inputs[bkey], np.float32)
        o = w.shape[1]
        f_cat = ci + HID
        wr = w.reshape(f_cat, M5, o)
        wmain = np.ascontiguousarray(np.concatenate([wr[ci:], wr[:ci]], axis=0))
        w0x = wr[:ci, 0, :]  # [ci, o]
        if ci == 2:
            wx = np.zeros((2 * BS, BS, o), np.float32)
            for b in range(BS):
                wx[2 * b, b, :] = w0x[0]
                wx[2 * b + 1, b, :] = w0x[1]
        elif ci == 1:
            wx = np.zeros((BS, BS, o), np.float32)
            for b in range(BS):
                wx[b, b, :] = w0x[0]
        else:
            wx = w0x.reshape(HID, 1, o)
        shared[f"w_{nm}"] = wmain
        shared[f"wx_{nm}"] = np.ascontiguousarray(wx)
        shared[f"b_{nm}"] = np.ascontiguousarray(bvec.reshape(o, 1))
    shared["pw"] = np.ascontiguousarray(np.asarray(inputs["p_w"], np.float32))
    shared["pb"] = np.ascontiguousarray(np.asarray(inputs["p_b"], np.float32).reshape(1, 1))
    return shared


def _prep_x(x, core, t_in):
    xc = np.asarray(x, np.float32)[:t_in, core * BS : (core + 1) * BS]  # (t, BS, N, 2)
    x_nm = np.ascontiguousarray(
        xc.reshape(t_in, BS, NT, 128, 2).transpose(0, 3, 2, 1, 4)
    )  # [t, p, kt, b, ci]
    x_fm = np.ascontiguousarray(
        xc.transpose(0, 1, 3, 2).reshape(t_in, 2 * BS, N)
    )  # rows 2b+ci
    return x_nm, x_fm


def _enable_trace_shim():
    """Register the NTFF profiling hook (axon) so trace=True works."""
    import types

    import antenv

    if "antenv.axon_hooks" in sys.modules:
        return
    mod = types.ModuleType("antenv.axon_hooks")
    state = {}
    mod.set_axon_ntff_profile_hook = lambda h: state.__setitem__("h", h)
    mod.get_axon_ntff_profile_hook = lambda: state.get("h")
    sys.modules["antenv.axon_hooks"] = mod
    antenv.axon_hooks = mod
    from trn_agent_boot.trn_boot import _ntff_profile_via_ctypes

    mod.set_axon_ntff_profile_hook(_ntff_profile_via_ctypes("/opt/axon/libaxon_pjrt.so"))
    import concourse.bass_utils as bu

    bu.upload_artifacts = lambda tmpdir: tmpdir


def run(inputs, t_in=T_IN, t_out=T_OUT, trace=False):
    global last_exec_ns
    from concourse.bass_utils import run_bass_kernel_spmd

    if trace:
        _enable_trace_shim()
    nc = _get_nc(t_in, t_out)
    shared = _prep_shared(inputs)
    in_maps = []
    for c in range(NCORES):
        x_nm_a, x_fm_a = _prep_x(inputs["x"], c, t_in)
        m = dict(shared)
        m["x_nm"] = x_nm_a
        m["x_fm"] = x_fm_a
        in_maps.append(m)
    res = run_bass_kernel_spmd(nc, in_maps, list(range(NCORES)), trace=trace)
    last_exec_ns = res.exec_time_ns
    out = np.concatenate([res.results[c]["y"] for c in range(NCORES)], axis=1)
    return np.ascontiguousarray(out.astype(np.float32))


def kernel(**inputs):
    return run(inputs, T_IN, T_OUT, trace=bool(os.environ.get("BASS_DCRNN_TRACE")))
